# revision 18
# baseline (speedup 1.0000x reference)
"""Trainium2 Bass kernel for a 2-layer LSTM (64, 32) + MLP head.

Model (PyTorch semantics, eval mode):
    h1 = LSTM(4 -> 64)(x)            x: [B=4096, T=512, 4]
    h2 = LSTM(64 -> 32)(h1)
    y  = (relu(h2[:, -1] @ w_fc1.T + b_fc1)) @ w_fc2.T + b_fc2   # [B, 1]

Sharding: data-parallel over batch across 8 NeuronCores (512 rows each),
weights replicated.

Per-core schedule:
  * State kept transposed and stacked: S [101, 256] per stream =
    [h1 (64); h2 (32); x_t (4); ones (1)].  Layer-1 and layer-2 are
    software-pipelined by one step and share the same rhs (layer-1's
    weight rows over h2 are zero, layer-2's over x are zero), so each
    gate needs ONE matmul (M stacked units, K=101) -- 4 MMs/step,
    with the x projection folded into the rhs (no separate x matmuls).
    Gate weights are padded to 128 columns (NumWeights==128 turns on
    the compiler's Fast Weight Load, halving LDWEIGHTS).
  * The batch 512 is split into TWO independent streams of 256 that
    run phase-shifted; this hides the per-step serial chain
    (MM -> sigmoid -> cell ops -> tanh -> h) behind the other stream.
  * ALL four gates use sigmoid: tanh(z) = 2*sigmoid(2z) - 1, with the
    g-gate weight block pre-scaled by 2.  One ACTIVATE covers i,f,g
    ([96, 768] across PSUM banks) on the critical path; the o gate's
    sigmoid is a separate op hidden under the DVE cell phase.  (ACT op
    cost ~= (N+350)/1.2 ns, so merging beats 5 per-gate ops by ~2x.)
  * Cell update: gfix = 2*sig_g-1 (tensor_scalar), then ONE paired
    tensor_tensor [m2|m1] = [sig_i|sig_f] (.) [gfix|C] followed by the
    add -- 4 DVE ops total per step including h = sig_o * tanh(C').
  * Junk matmuls on constant operands keep the PE's HAM clock-gate at
    8/8 (2.4 GHz); without them the bursty MM pattern runs at 1.2 GHz.
  * x_t arrives by DMA into rows 96:100 of the next S tile (3-deep
    rotation per stream) two steps ahead.
"""

import numpy as np
from contextlib import ExitStack

import concourse.bass as bass
import concourse.tile as tile
from concourse import bacc, mybir
from concourse import bass_utils

AF = mybir.ActivationFunctionType
ALU = mybir.AluOpType

B, T, D_IN, H1, H2 = 4096, 512, 4, 64, 32
NCORES = 8
BL = B // NCORES        # 512 batch rows per core
NSTREAM = 2
SL = BL // NSTREAM      # 256 batch rows per stream

F32 = mybir.dt.float32
DT = mybir.dt.bfloat16

HS = H1 + H2            # 96 stacked units
KS = HS + D_IN + 1      # 101 rhs rows: h1|h2|x|ones
XROW = HS               # 96: first x row
ONEROW = HS + D_IN      # 100: ones row
R = 3                   # S-tile rotation depth per stream

# gate order in the fused weight/psum layout
GATES = ("i", "f", "g", "o")


def _build(n_steps: int = T):
    nc = bacc.Bacc("TRN2", target_bir_lowering=False, debug=False)

    xT = nc.dram_tensor("xT", [n_steps * 4, BL], DT, kind="ExternalInput")
    # gate weights padded to 128 columns each (Fast Weight Load needs
    # NumWeights==128; PSUM rows 96:128 are garbage and never read)
    w12 = nc.dram_tensor("w12", [KS, 4 * 128], DT, kind="ExternalInput")
    wf1 = nc.dram_tensor("wf1", [KS, 16], DT, kind="ExternalInput")
    wf2 = nc.dram_tensor("wf2", [16, 1], DT, kind="ExternalInput")
    bf2 = nc.dram_tensor("bf2", [1, 1], F32, kind="ExternalInput")
    out = nc.dram_tensor("out", [1, BL], F32, kind="ExternalOutput")

    with tile.TileContext(nc) as tc, ExitStack() as ctx:
        const = ctx.enter_context(tc.tile_pool(name="const", bufs=1))
        gtp = ctx.enter_context(tc.tile_pool(name="gt", bufs=3))
        scr = ctx.enter_context(tc.tile_pool(name="scr", bufs=6))

        W12 = const.tile([KS, 4 * 128], DT, tag="W12")
        nc.sync.dma_start(W12[:], w12.ap())
        WF1 = const.tile([KS, 16], DT, tag="WF1")
        nc.sync.dma_start(WF1[:], wf1.ap())
        WF2 = const.tile([16, 1], DT, tag="WF2")
        nc.sync.dma_start(WF2[:], wf2.ap())
        BF2 = const.tile([1, 1], F32, tag="BF2")
        nc.sync.dma_start(BF2[:], bf2.ap())

        # Per-stream persistent state.  GC packs [gfix | C] side by side so
        # the two cell products run as ONE paired tensor_tensor:
        #   [m2|m1] = [sig_i|sig_f] (.) [gfix|C]
        S = [[const.tile([KS, SL], DT, name=f"S{s}_{r}", tag=f"S{s}_{r}")
              for r in range(R)] for s in range(NSTREAM)]
        GC = [const.tile([HS, 2 * SL], DT, name=f"GC{s}", tag=f"GC{s}")
              for s in range(NSTREAM)]
        for s in range(NSTREAM):
            for r in range(R):
                nc.vector.memset(S[s][r][:], 0.0)
                # base partition must be 32-aligned: set rows 96:101 to one;
                # the x DMA overwrites rows 96:100 before every use.
                nc.vector.memset(S[s][r][XROW:KS, :], 1.0)
            nc.vector.memset(GC[s][:], 0.0)

        def dma_x(s, k):
            # x_k for stream s into rows 96:100 of S[s][k%R]
            if k < n_steps:
                nc.sync.dma_start(
                    S[s][k % R][XROW : XROW + D_IN, :],
                    xT.ap()[4 * k : 4 * k + 4, s * SL : (s + 1) * SL],
                )

        with tc.tile_pool(name="psum0", bufs=1, space="PSUM") as psum0, \
             tc.tile_pool(name="psum1", bufs=1, space="PSUM") as psum1, \
             tc.tile_pool(name="psumj", bufs=1, space="PSUM") as psumj:
            psums = [psum0, psum1]
            PJ = psumj.tile([128, 512], F32, tag="PJ")
            for s in range(NSTREAM):
                dma_x(s, 0)
                dma_x(s, 1)

            for k in range(n_steps + 1):
                for s in range(NSTREAM):
                    Scur = S[s][k % R]
                    Snxt = S[s][(k + 1) % R]
                    # junk matmuls (constant operands, dead output) keep the
                    # PE HAM clock-gate at 8/8 between real bursts; the real
                    # matmuls then stream at 2.4 GHz instead of 1.2.  They sit
                    # BEFORE the real ones so they drain while the PE queue
                    # waits for h.
                    for _ in range(3):
                        nc.tensor.matmul(PJ[:, 0:256], W12[:, 0:128],
                                         W12[:, 0:256], start=True, stop=True)
                    P = psums[s].tile([128, 4 * SL], F32, tag=f"P{s}")
                    for g in range(4):
                        nc.tensor.matmul(
                            P[:, g * SL : (g + 1) * SL],
                            W12[:, g * 128 : (g + 1) * 128],
                            Scur[:],
                            start=True,
                            stop=True,
                        )
                    # sigmoid over i,f,g on the critical path; the o gate is
                    # only needed for the final h product, so its sigmoid is
                    # a separate op that hides under the DVE cell phase.
                    GT = gtp.tile([HS, 4 * SL], DT, tag=f"GT{s}")
                    nc.scalar.activation(GT[:, 0 : 3 * SL], P[0:HS, 0 : 3 * SL],
                                         AF.Sigmoid)
                    nc.scalar.activation(GT[:, 3 * SL : 4 * SL],
                                         P[0:HS, 3 * SL : 4 * SL], AF.Sigmoid)
                    SGO = GT[:, 3 * SL : 4 * SL]
                    Ccur = GC[s][:, SL : 2 * SL]

                    # gfix = 2*sigmoid(2 z_g) - 1 = tanh(z_g)  (w pre-scaled)
                    nc.vector.tensor_scalar(GC[s][:, 0:SL], GT[:, 2 * SL : 3 * SL],
                                            2.0, -1.0, ALU.mult, ALU.add)
                    # [m2|m1] = [sig_i|sig_f] (.) [gfix|C]   (one 512-col TT)
                    M12 = scr.tile([HS, 2 * SL], DT, tag=f"M12{s}")
                    nc.vector.tensor_tensor(M12[:], GT[:, 0 : 2 * SL], GC[s][:],
                                            ALU.mult)
                    # C' = m2 + m1
                    nc.vector.tensor_tensor(Ccur, M12[:, 0:SL], M12[:, SL : 2 * SL],
                                            ALU.add)
                    TC = scr.tile([HS, SL], DT, tag=f"TC{s}")
                    nc.scalar.activation(TC[:], Ccur, AF.Tanh)
                    nc.vector.tensor_tensor(Snxt[0:HS, :], SGO, TC[:],
                                            ALU.mult)
                    if k == 0:
                        # wipe garbage layer-2 state from pipeline warmup
                        nc.vector.memset(Snxt[H1:HS, :], 0.0)
                        nc.vector.memset(GC[s][H1:HS, SL : 2 * SL], 0.0)
                    dma_x(s, k + 2)

        # MLP head on h2 of the final state tiles
        with tc.tile_pool(name="psh", bufs=1, space="PSUM") as psh:
            for s in range(NSTREAM):
                Sfin = S[s][(n_steps + 1) % R]
                PF = psh.tile([16, SL], F32, tag=f"PF{s}")
                nc.tensor.matmul(PF[:], WF1[:, :], Sfin[:], start=True,
                                 stop=True)
                Z = scr.tile([16, SL], DT, tag=f"Z{s}")
                nc.scalar.activation(Z[:], PF[:], AF.Relu)
                PO = psh.tile([1, SL], F32, tag=f"PO{s}")
                nc.tensor.matmul(PO[:], WF2[:, :], Z[:], start=True, stop=True)
                Y = scr.tile([1, SL], F32, tag=f"Y{s}")
                nc.scalar.activation(Y[:], PO[:], AF.Identity,
                                     bias=BF2[:, 0:1])
                nc.sync.dma_start(out.ap()[:, s * SL : (s + 1) * SL], Y[:])

    nc.compile()
    return nc


def _pack_weights(inputs, np_dt):
    w_ih1, w_hh1 = inputs["w_ih1"], inputs["w_hh1"]
    w_ih2, w_hh2 = inputs["w_ih2"], inputs["w_hh2"]
    b1 = (inputs["b_ih1"] + inputs["b_hh1"]).astype(np.float32)
    b2 = (inputs["b_ih2"] + inputs["b_hh2"]).astype(np.float32)

    w12 = np.zeros((KS, 4 * 128), np.float32)
    for g in range(4):
        scale = 2.0 if g == 2 else 1.0  # g-gate: tanh(z) = 2 sig(2z) - 1
        c0 = g * 128
        # layer-1 units: cols c0 : c0+64
        w12[0:H1, c0 : c0 + H1] = w_hh1[g * H1 : (g + 1) * H1, :].T * scale
        w12[XROW : XROW + D_IN, c0 : c0 + H1] = (
            w_ih1[g * H1 : (g + 1) * H1, :].T * scale
        )
        w12[ONEROW, c0 : c0 + H1] = b1[g * H1 : (g + 1) * H1] * scale
        # layer-2 units: cols c0+64 : c0+96
        w12[0:H1, c0 + H1 : c0 + HS] = w_ih2[g * H2 : (g + 1) * H2, :].T * scale
        w12[H1:HS, c0 + H1 : c0 + HS] = w_hh2[g * H2 : (g + 1) * H2, :].T * scale
        w12[ONEROW, c0 + H1 : c0 + HS] = b2[g * H2 : (g + 1) * H2] * scale

    wf1 = np.zeros((KS, 16), np.float32)
    wf1[H1:HS, :] = inputs["w_fc1"].T
    wf1[ONEROW, :] = inputs["b_fc1"]

    return {
        "w12": np.ascontiguousarray(w12).astype(np_dt),
        "wf1": np.ascontiguousarray(wf1).astype(np_dt),
        "wf2": np.ascontiguousarray(inputs["w_fc2"].T).astype(np_dt),
        "bf2": np.ascontiguousarray(inputs["b_fc2"][:, None]).astype(np.float32),
    }


_built = {}


def _get_nc(n_steps):
    if n_steps not in _built:
        _built[n_steps] = _build(n_steps)
    return _built[n_steps]


def _run(inputs, n_steps=T, **run_kwargs):
    np_dt = mybir.dt.np(DT)
    x = np.asarray(inputs["x"], np.float32)
    nb = x.shape[0]
    ncores = NCORES
    bl = nb // ncores
    assert bl == BL and x.shape[1] >= n_steps
    shared = _pack_weights(
        {k: np.asarray(v, np.float32) for k, v in inputs.items() if k != "x"},
        np_dt,
    )
    in_maps = []
    for c in range(ncores):
        xs = x[c * bl : (c + 1) * bl, :n_steps, :]  # [BL, T, 4]
        xTc = np.ascontiguousarray(
            xs.transpose(1, 2, 0).reshape(n_steps * 4, bl)
        )
        in_maps.append(dict(shared, xT=xTc.astype(np_dt)))
    nc = _get_nc(n_steps)
    res = bass_utils.run_bass_kernel_spmd(
        nc, in_maps, core_ids=list(range(ncores)), **run_kwargs
    )
    y = np.concatenate(
        [np.asarray(r["out"], np.float32).reshape(bl, 1) for r in res.results],
        axis=0,
    )
    return y, res


def kernel(**inputs) -> np.ndarray:
    y, _ = _run(inputs)
    return y
